# revision 5
# baseline (speedup 1.0000x reference)
"""Trainium2 Bass kernel for ConcatenateSphericalSignals.

The op: concat(signal1, signal2) along the channel dim, then apply a
768x768 one-hot permutation matrix to the channel dim (einsum
'dc,ncba->ndba').  The mixing matrix merge-sorts contiguous channel
blocks, so the whole op collapses to a few large contiguous block
copies per sample.  We shard the batch dim N=16 across 8 cores (2
samples/core) and issue one flat DRAM->DRAM DMA per (sample, block).

A flat 1D access pattern is essential: balance_dma_aps splits a
single-dim AP into 64KiB rows with a 16-multiple row count, and the
descriptor generator sprays rows across all 16 SDMA engines.  Higher-
rank APs spray only over the outermost dim (e.g. [2, ...] -> 2 engines),
which is 3-5x slower.  Measured: ~90-105 us/core for 24 MiB copied
(~330 GB/s/core), at the per-engine DRAM->DRAM throughput limit.
"""

import numpy as np

import concourse.bass as bass
import concourse.mybir as mybir
from concourse.bass_utils import run_bass_kernel_spmd

# Problem shape (hardcoded per harness contract).
N, F1, F2 = 16, 288, 480
FO = F1 + F2
B, A = 64, 64
BA = B * A
NCORES = 8
NLOC = N // NCORES  # samples per core

# Test harness hooks: set TRACE=True before calling kernel() to collect a
# profile; LAST_RESULT then holds the BassKernelResults.
TRACE = False
LAST_RESULT = None

_module_cache: dict = {}


def _copy_plan(mixing_matrix: np.ndarray):
    """Decompose a one-hot permutation matrix into maximal contiguous
    block copies (src_tensor_idx, src_chan_start, dst_chan_start, length).
    Returns None if the matrix is not a one-hot permutation."""
    M = np.asarray(mixing_matrix)
    if M.shape != (FO, FO):
        return None
    perm = M.argmax(axis=1).astype(np.int64)
    if not np.array_equal(np.sort(perm), np.arange(FO)):
        return None
    ref = np.zeros(M.shape, dtype=M.dtype)
    ref[np.arange(FO), perm] = 1
    if not np.array_equal(ref, M):
        return None

    runs = []
    d = 0
    while d < FO:
        c0 = int(perm[d])
        L = 1
        while (
            d + L < FO
            and int(perm[d + L]) == c0 + L
            and (c0 < F1) == (c0 + L < F1)  # stay within one source tensor
        ):
            L += 1
        if c0 < F1:
            runs.append((0, c0, d, L))
        else:
            runs.append((1, c0 - F1, d, L))
        d += L
    return tuple(runs)


def _build_module(runs):
    nc = bass.Bass()
    s1 = nc.declare_dram_parameter(
        "signal1", [NLOC, F1, BA], mybir.dt.float32, isOutput=False
    )
    s2 = nc.declare_dram_parameter(
        "signal2", [NLOC, F2, BA], mybir.dt.float32, isOutput=False
    )
    out = nc.declare_dram_parameter(
        "out", [NLOC, FO, BA], mybir.dt.float32, isOutput=True
    )
    srcs = [s1, s2]
    with nc.Block() as block, nc.semaphore("dma_sem") as dma_sem:

        @block.gpsimd
        def _(gpsimd):
            ndma = 0
            for which, c0, d0, L in runs:
                for n in range(NLOC):
                    gpsimd.dma_start(
                        out=out[n, d0 : d0 + L, :].rearrange("c f -> (c f)"),
                        in_=srcs[which][n, c0 : c0 + L, :].rearrange(
                            "c f -> (c f)"
                        ),
                    ).then_inc(dma_sem, 16)
                    ndma += 1
            gpsimd.wait_ge(dma_sem, 16 * ndma)

    return nc


def kernel(signal1: np.ndarray, signal2: np.ndarray, mixing_matrix: np.ndarray):
    global LAST_RESULT
    signal1 = np.ascontiguousarray(np.asarray(signal1, dtype=np.float32))
    signal2 = np.ascontiguousarray(np.asarray(signal2, dtype=np.float32))
    assert signal1.shape == (N, F1, B, A)
    assert signal2.shape == (N, F2, B, A)

    runs = _copy_plan(mixing_matrix)
    if runs is None:
        # Defensive fallback (never hit for the reference module, whose
        # buffer is a one-hot permutation by construction).
        combined = np.concatenate([signal1, signal2], axis=1)
        return np.einsum(
            "dc,ncba->ndba", np.asarray(mixing_matrix, np.float32), combined
        )

    nc = _module_cache.get(runs)
    if nc is None:
        nc = _build_module(runs)
        _module_cache[runs] = nc

    s1 = signal1.reshape(N, F1, BA)
    s2 = signal2.reshape(N, F2, BA)
    core_ids = list(range(NCORES))
    in_maps = [
        {
            "signal1": s1[c * NLOC : (c + 1) * NLOC],
            "signal2": s2[c * NLOC : (c + 1) * NLOC],
        }
        for c in core_ids
    ]

    res = None
    last_exc = None
    for _attempt in range(3):
        try:
            res = run_bass_kernel_spmd(nc, in_maps, core_ids, trace=TRACE)
            break
        except Exception as e:  # rare transient NRT_EXEC_UNIT_UNRECOVERABLE
            last_exc = e
    if res is None:
        raise last_exc
    LAST_RESULT = res

    out = np.concatenate([r["out"] for r in res.results], axis=0)
    return out.reshape(N, FO, B, A)


# revision 7
# speedup vs baseline: 1.1639x; 1.1639x over previous
"""Trainium2 Bass kernel for ConcatenateSphericalSignals.

The op: concat(signal1, signal2) along the channel dim, then apply a
768x768 one-hot permutation matrix to the channel dim (einsum
'dc,ncba->ndba').  The mixing matrix merge-sorts contiguous channel
blocks, so the whole op collapses to a few large contiguous block
copies per sample.  We shard the batch dim N=16 across 8 cores (2
samples/core) and issue one flat DRAM->DRAM DMA per (sample, block).

A flat 1D access pattern is essential: balance_dma_aps splits a
single-dim AP into 64KiB rows with a 16-multiple row count, and the
descriptor generator sprays rows across all 16 SDMA engines.  Higher-
rank APs spray only over the outermost dim (e.g. [2, ...] -> 2 engines),
which is 3-5x slower.  Issuing from the scalar engine (ACT HWDGE ring)
starts ~1.3us earlier than gpsimd (whose SWDGE work queues behind the
Bass preamble's sem-clears/memsets that run on gpsimd).  Measured:
~88-105 us/core for 24 MiB copied (~330 GB/s/core), at the per-engine
DRAM->DRAM duplex throughput limit.
"""

import numpy as np

import concourse.bass as bass
import concourse.mybir as mybir
from concourse.bass_utils import run_bass_kernel_spmd

# Problem shape (hardcoded per harness contract).
N, F1, F2 = 16, 288, 480
FO = F1 + F2
B, A = 64, 64
BA = B * A
NCORES = 8
NLOC = N // NCORES  # samples per core

# Test harness hooks: set TRACE=True before calling kernel() to collect a
# profile; LAST_RESULT then holds the BassKernelResults.
TRACE = False
LAST_RESULT = None

_module_cache: dict = {}


def _copy_plan(mixing_matrix: np.ndarray):
    """Decompose a one-hot permutation matrix into maximal contiguous
    block copies (src_tensor_idx, src_chan_start, dst_chan_start, length).
    Returns None if the matrix is not a one-hot permutation."""
    M = np.asarray(mixing_matrix)
    if M.shape != (FO, FO):
        return None
    perm = M.argmax(axis=1).astype(np.int64)
    if not np.array_equal(np.sort(perm), np.arange(FO)):
        return None
    ref = np.zeros(M.shape, dtype=M.dtype)
    ref[np.arange(FO), perm] = 1
    if not np.array_equal(ref, M):
        return None

    runs = []
    d = 0
    while d < FO:
        c0 = int(perm[d])
        L = 1
        while (
            d + L < FO
            and int(perm[d + L]) == c0 + L
            and (c0 < F1) == (c0 + L < F1)  # stay within one source tensor
        ):
            L += 1
        if c0 < F1:
            runs.append((0, c0, d, L))
        else:
            runs.append((1, c0 - F1, d, L))
        d += L
    return tuple(runs)


def _build_module(runs):
    nc = bass.Bass()
    s1 = nc.declare_dram_parameter(
        "signal1", [NLOC, F1, BA], mybir.dt.float32, isOutput=False
    )
    s2 = nc.declare_dram_parameter(
        "signal2", [NLOC, F2, BA], mybir.dt.float32, isOutput=False
    )
    out = nc.declare_dram_parameter(
        "out", [NLOC, FO, BA], mybir.dt.float32, isOutput=True
    )
    srcs = [s1, s2]
    with nc.Block() as block, nc.semaphore("dma_sem") as dma_sem:

        @block.scalar
        def _(scalar):
            ndma = 0
            for which, c0, d0, L in runs:
                for n in range(NLOC):
                    scalar.dma_start(
                        out=out[n, d0 : d0 + L, :].rearrange("c f -> (c f)"),
                        in_=srcs[which][n, c0 : c0 + L, :].rearrange(
                            "c f -> (c f)"
                        ),
                    ).then_inc(dma_sem, 16)
                    ndma += 1
            scalar.wait_ge(dma_sem, 16 * ndma)

    return nc


def kernel(signal1: np.ndarray, signal2: np.ndarray, mixing_matrix: np.ndarray):
    global LAST_RESULT
    signal1 = np.ascontiguousarray(np.asarray(signal1, dtype=np.float32))
    signal2 = np.ascontiguousarray(np.asarray(signal2, dtype=np.float32))
    assert signal1.shape == (N, F1, B, A)
    assert signal2.shape == (N, F2, B, A)

    runs = _copy_plan(mixing_matrix)
    if runs is None:
        # Defensive fallback (never hit for the reference module, whose
        # buffer is a one-hot permutation by construction).
        combined = np.concatenate([signal1, signal2], axis=1)
        return np.einsum(
            "dc,ncba->ndba", np.asarray(mixing_matrix, np.float32), combined
        )

    nc = _module_cache.get(runs)
    if nc is None:
        nc = _build_module(runs)
        _module_cache[runs] = nc

    s1 = signal1.reshape(N, F1, BA)
    s2 = signal2.reshape(N, F2, BA)
    core_ids = list(range(NCORES))
    in_maps = [
        {
            "signal1": s1[c * NLOC : (c + 1) * NLOC],
            "signal2": s2[c * NLOC : (c + 1) * NLOC],
        }
        for c in core_ids
    ]

    res = None
    last_exc = None
    for _attempt in range(3):
        try:
            res = run_bass_kernel_spmd(nc, in_maps, core_ids, trace=TRACE)
            break
        except Exception as e:  # rare transient NRT_EXEC_UNIT_UNRECOVERABLE
            last_exc = e
    if res is None:
        raise last_exc
    LAST_RESULT = res

    out = np.concatenate([r["out"] for r in res.results], axis=0)
    return out.reshape(N, FO, B, A)
